# revision 1
# baseline (speedup 1.0000x reference)
"""Multi-head attention forward on 8 TRN2 NeuronCores, data-parallel over batch.

Reference computation (per batch element b):
    qkv  = x @ qkv_w.T + qkv_b                     # [N, 3D]
    q, k = LN_headdim(q), LN_headdim(k)            # layernorm over head_dim=64
    S    = q @ k.T * hd^-0.5 ; A = softmax_j(S)    # per head
    out  = (A @ v) @ proj_w.T + proj_b             # [N, D]

Design (one batch element per core, no collectives). Engine split aims for a
dense TensorE stream with ScalarE free for the softmax exps:
  - Host-side layout prep: x / weights transposed + bf16-cast on the CPU so
    DMAs land straight in contraction-on-partitions SBUF layouts.
  - qkv bias is applied during PSUM evacuation by VectorE (tensor_tensor add
    against a DMA-broadcast bias tile) -- no bias matmuls on TensorE.
  - LN group MEANS come free from TensorE: host-precomputed -mean weight
    columns (wsum) ride the qkv matmul; only the sum-of-squares needs an
    on-core reduce (GpSimd squares bf16 t, one grouped VectorE reduce).
  - q normalize runs on ScalarE (idle until the first exp) as
    Identity(t*rstd + (-mu*rstd)) with per-partition scale/bias APs.
  - k is only CENTERED (add -mu, GpSimd); its rstd and the 1/sqrt(hd) scale
    fold into the exp ACTIVATE's per-partition scale AP (scores PSUM has
    k-tokens on partitions).
  - q is stored in two zero-padded token-major buffers so a plain 128x128
    block DMA transpose directly yields the K=128 zero-padded rhs operands
    (full-height matmuls keep the PE HAM clock-gate warm); no on-core copies.
  - Scores computed TRANSPOSED: E = exp(scale_k * (k_c . q_n)) lands with
    k-tokens on partitions, directly the rhs of attn@v with V as lhsT.
  - Softmax denominators via 64 ones-columns in V (PSUM rows 64:128 hold the
    sums); normalize = SBUF copy + reciprocal_approx_fast + one VectorE
    multiply writing attnoutT.  (reciprocal_approx_fast must NOT read PSUM
    directly: it returns garbage on HW while passing in CoreSim.)
  - Heads software-pipelined 1:1 (scores of h interleave attn@v of h-1);
    the v-chunk matmuls fill TensorE gaps during the first exp drains.
  - Projection computes outT = projwT.T @ attnoutT; ScalarE (idle after exps)
    applies the per-partition bias during PSUM evacuation; host flips back.
"""

import os
import sys

import numpy as np

sys.path.insert(0, "/opt/trn_rl_repo")

# NOTE: reciprocal_approx_fast reading PSUM directly returns garbage on HW
# (works in CoreSim); always stage denominators through SBUF first.

from contextlib import ExitStack

import concourse.bass as bass
import concourse.tile as tile
from concourse import bacc, mybir
from concourse.bass_utils import run_bass_kernel_spmd

B, N, D = 8, 1024, 768
H, HD = 12, 64
O3 = 3 * D  # 2304
P = 128
NT = N // P  # 8 token tiles
DC = D // P  # 6 contraction subtiles
EPS = 1e-5
SCALE = HD ** -0.5  # 0.125
F32 = mybir.dt.float32
BF16 = mybir.dt.bfloat16


def _bcast_ap(ap_1d, parts):
    """View a 1-D DRAM AP as [parts, n] with partition stride 0 (broadcast)."""
    return bass.AP(
        tensor=ap_1d.tensor,
        offset=ap_1d.offset,
        ap=[[0, parts]] + list(ap_1d.ap),
    )


def _build_graph(apply_gn):
    nc = bacc.Bacc("TRN2", target_bir_lowering=False, debug=False, num_devices=B)

    x_d = nc.dram_tensor("x", [D, N], BF16, kind="ExternalInput").ap()
    qkvw_d = nc.dram_tensor("qkv_w", [D, O3], BF16, kind="ExternalInput").ap()
    qkvb_d = nc.dram_tensor("qkv_b", [O3], F32, kind="ExternalInput").ap()
    qkvbh_d = nc.dram_tensor("qkv_bh", [O3], BF16, kind="ExternalInput").ap()
    # host-precomputed -mean columns: wsum[:, g] = -mean of qkv_w rows of
    # head-group g (24 q+k groups); bsum[g] = -mean of the group's bias
    wsum_d = nc.dram_tensor("qkv_wsum", [D, 24], BF16, kind="ExternalInput").ap()
    bsum_d = nc.dram_tensor("qkv_bsum", [24], F32, kind="ExternalInput").ap()
    projw_d = nc.dram_tensor("proj_w", [D, D], BF16, kind="ExternalInput").ap()
    projb_d = nc.dram_tensor("proj_b", [D], F32, kind="ExternalInput").ap()
    gamma_d = nc.dram_tensor("qn_gamma", [HD], F32, kind="ExternalInput").ap()
    beta_d = nc.dram_tensor("qn_beta", [HD], F32, kind="ExternalInput").ap()
    # output is produced TRANSPOSED ([e, t]); the host flips it back
    out_d = nc.dram_tensor("out", [D, N], F32, kind="ExternalOutput").ap()

    with tile.TileContext(nc) as tc:
        _emit(tc, out_d, x_d, qkvw_d, qkvb_d, qkvbh_d, wsum_d, bsum_d,
              projw_d, projb_d, gamma_d, beta_d, apply_gn)

    nc.compile()
    return nc


def _emit(tc, out_d, x_d, qkvw_d, qkvb_d, qkvbh_d, wsum_d, bsum_d, projw_d,
          projb_d, gamma_d, beta_d, apply_gn):
    nc = tc.nc
    ctx = ExitStack()
    with ctx:
        const = ctx.enter_context(tc.tile_pool(name="const", bufs=1))
        wpool = ctx.enter_context(tc.tile_pool(name="wts", bufs=1))
        data = ctx.enter_context(tc.tile_pool(name="data", bufs=1))
        epool = ctx.enter_context(tc.tile_pool(name="escore", bufs=2))
        qkpool = ctx.enter_context(tc.tile_pool(name="qk", bufs=2))
        tpool = ctx.enter_context(tc.tile_pool(name="tev", bufs=4))
        spool = ctx.enter_context(tc.tile_pool(name="stats", bufs=3))
        outp = ctx.enter_context(tc.tile_pool(name="outp", bufs=3))
        nrm = ctx.enter_context(tc.tile_pool(name="nrm", bufs=1))

        # ---- DMA x and weights straight into [k, ., m] SBUF layouts ----
        # interleave x/qkv-w per dc so the first matmuls can start early;
        # the (large) broadcast constants are issued AFTER the tensors the
        # first matmuls need -- they are not read until the first evacuation
        xT = wpool.tile([P, DC, N], BF16)      # [d_in, d_out, t]
        qkvwT = wpool.tile([P, DC, O3], BF16)  # [d_in, d_out, o]
        wsumT = wpool.tile([P, DC, 24], BF16)  # [d_in, d_out, group]
        projwT = wpool.tile([P, DC, D], BF16)  # [o_in, o_out, e]
        for dc in range(DC):
            nc.sync.dma_start(
                xT[:, dc, :],
                x_d.rearrange("(dc p) t -> p dc t", p=P)[:, dc, :],
            )
            nc.sync.dma_start(
                wsumT[:, dc, :],
                wsum_d.rearrange("(dc p) g -> p dc g", p=P)[:, dc, :],
            )
            nc.sync.dma_start(
                qkvwT[:, dc, :],
                qkvw_d.rearrange("(dc p) o -> p dc o", p=P)[:, dc, :],
            )

        # ---- constants (after the matmul operands in DMA order) ----
        bias_bc = const.tile([P, O3], BF16)
        nc.sync.dma_start(bias_bc[:], _bcast_ap(qkvbh_d, P))
        mub_bc = const.tile([P, 24], F32)
        nc.sync.dma_start(mub_bc[:], _bcast_ap(bsum_d, P))
        projb_col = const.tile([P, DC], F32)
        nc.sync.dma_start(projb_col[:], projb_d.rearrange("(et p) -> p et", p=P))
        if apply_gn:
            gamma_bc = const.tile([P, HD], F32)
            nc.sync.dma_start(gamma_bc[:], _bcast_ap(gamma_d, P))
            beta_bc = const.tile([P, HD], F32)
            nc.sync.dma_start(beta_bc[:], _bcast_ap(beta_d, P))

        for dc in range(DC):
            nc.sync.dma_start(
                projwT[:, dc, :],
                projw_d.rearrange("(dc p) e -> p dc e", p=P)[:, dc, :],
            )

        # q stored token-major in two half-zero buffers: a plain 128x128 block
        # DMA transpose of qnp0[:, hp] directly yields the zero-padded rhs for
        # the even head of pair hp (and qnp1 the odd head).
        qnp0 = data.tile([P, DC, NT, P], BF16)  # cols 0:64 = q even head
        qnp1 = data.tile([P, DC, NT, P], BF16)  # cols 64:128 = q odd head
        nc.vector.memset(qnp0[:, :, :, HD:2 * HD], 0.0)
        nc.vector.memset(qnp1[:, :, :, 0:HD], 0.0)
        # k centered (not scaled), stored per head-pair: [t, pair, t_out, 128]
        knp = data.tile([P, DC, NT, P], BF16)
        # v with 64 ones-columns: attn@v psum rows 64:128 become the softmax
        # denominator s[i], broadcast across 64 partitions by the PE for free
        vext = data.tile([P, NT, H, 2 * HD], BF16)
        nc.vector.memset(vext[:, :, :, HD:2 * HD], 1.0)
        # attnoutT [o_in, o_out, t] written directly by the normalize step
        attnoutT = data.tile([P, DC, N], BF16)
        # 0.125 * rstd_k per (token-tile, head): per-partition exp scales
        rks = data.tile([P, NT, H], F32)

        # ---- QKV projection (q,k) + bias + head-dim layernorm ----
        # Two column passes so the softmax exp stream can start after ~25% of
        # the QKV work: pass A computes q,k for head-pairs 0-1 (all token
        # tiles) -> heads 0-3 attention starts; pass B (heads 4-11 columns)
        # runs underneath the exp stream.
        # Engine split: DVE evacuates (+bias) and reduces sum-of-squares;
        # GpSimd squares and centers k; the per-head q normalize
        # Identity(t*rstd + (-mu*rstd)) runs on ScalarE in pass A (ScalarE
        # idle before the first exp) and on DVE in pass B (ScalarE busy).
        # Group means come free from TensorE via the wsum columns.
        # Group order (wsum cols / mu_all / stats): A = [q0-3, k0-3],
        # B = [q4-11, k4-11].
        mu_all = data.tile([P, NT, 24], F32)

        def emit_stats(tt, stats, g0, nG, rks_lo, rks_hi):
            """var/rstd/m2 from sumsq `stats` and mu_all groups [g0, g0+nG);
            k-group rstds scaled into the exp-scale table rks."""
            mu = mu_all[:, tt, g0:g0 + nG]
            var = spool.tile([P, 24], F32, tag="var", name="var")[:, :nG]
            nc.vector.tensor_scalar(var, stats[:, :nG], 1.0 / HD, EPS,
                                    op0=mybir.AluOpType.mult,
                                    op1=mybir.AluOpType.add)
            msq = spool.tile([P, 24], F32, tag="msq", name="msq")[:, :nG]
            nc.vector.tensor_tensor(msq, mu, mu, op=mybir.AluOpType.mult)
            nc.vector.tensor_tensor(var, var, msq,
                                    op=mybir.AluOpType.subtract)
            std = spool.tile([P, 24], F32, tag="sd", name="std")[:, :nG]
            nc.scalar.activation(std, var,
                                 mybir.ActivationFunctionType.Sqrt)
            rstd = spool.tile([P, 24], F32, tag="rs", name="rstd")[:, :nG]
            nc.vector.reciprocal_approx_fast(rstd, std)
            m2 = spool.tile([P, 24], F32, tag="m2", name="m2")[:, :nG]
            nc.vector.tensor_tensor(m2, mu, rstd, op=mybir.AluOpType.mult)
            # k-group slice of this pass -> exp scales (0.125 * rstd_k)
            nc.vector.tensor_scalar_mul(
                rks[:, tt, rks_lo:rks_hi], rstd[:, nG // 2:nG], SCALE)
            return rstd, m2

        def emit_qnorm(tt, h, src, rstd, m2, j, on_scalar):
            hp, odd = divmod(h, 2)
            dst = (qnp1[:, hp, tt, HD:2 * HD] if odd
                   else qnp0[:, hp, tt, 0:HD])
            if on_scalar:
                nc.scalar.activation(
                    dst, src, mybir.ActivationFunctionType.Identity,
                    bias=m2[:, j:j + 1], scale=rstd[:, j:j + 1])
            else:
                nc.vector.tensor_scalar(
                    dst, src, rstd[:, j:j + 1], m2[:, j:j + 1],
                    op0=mybir.AluOpType.mult, op1=mybir.AluOpType.add)
            if apply_gn:
                nc.gpsimd.tensor_tensor(dst, dst, gamma_bc[:, 0:HD],
                                        op=mybir.AluOpType.mult)
                nc.gpsimd.tensor_tensor(dst, dst, beta_bc[:, 0:HD],
                                        op=mybir.AluOpType.add)

        def emit_kcenter(tt, kp0, nkp, src, mu_g0, rstd, m2, j0):
            # non-gn: center only (add -mu); gn: full LN + gamma/beta
            if not apply_gn:
                dst = knp[:, 2 * kp0:2 * (kp0 + nkp), tt, :].rearrange(
                    "p a (s h) -> p a s h", h=HD)
                mub = mu_all[:, tt, mu_g0:mu_g0 + 4 * nkp].rearrange(
                    "p (a s) -> p a s", s=2)[
                    :, :, :, None].to_broadcast((P, 2 * nkp, 2, HD))
                nc.gpsimd.tensor_tensor(
                    dst, src.rearrange("p (a s h) -> p a s h", s=2, h=HD),
                    mub, op=mybir.AluOpType.add)
            else:
                for g in range(4 * nkp):
                    h = 4 * kp0 + g
                    hp, odd = divmod(h, 2)
                    dst = knp[:, hp, tt, odd * HD:(odd + 1) * HD]
                    nc.vector.tensor_scalar(
                        dst, src[:, g * HD:(g + 1) * HD],
                        rstd[:, j0 + g:j0 + g + 1], m2[:, j0 + g:j0 + g + 1],
                        op0=mybir.AluOpType.mult, op1=mybir.AluOpType.add)
                    nc.gpsimd.tensor_tensor(dst, dst, gamma_bc[:, 0:HD],
                                            op=mybir.AluOpType.mult)
                    nc.gpsimd.tensor_tensor(dst, dst, beta_bc[:, 0:HD],
                                            op=mybir.AluOpType.add)

        with tc.tile_pool(name="ps_qk", bufs=2, space="PSUM") as ps_qk:
            for tt in range(NT):
                pmu = ps_qk.tile([P, 24], F32, tag="mu", name="ps_mu")
                psum = ps_qk.tile([P, 3 * 512], F32, tag="qk", name="ps_qk")
                for dc in range(DC):
                    lhsT = xT[:, dc, tt * P:(tt + 1) * P]
                    nc.tensor.matmul(pmu, lhsT=lhsT, rhs=wsumT[:, dc, :],
                                     start=(dc == 0), stop=(dc == DC - 1))
                    for sg in range(3):
                        nc.tensor.matmul(
                            psum[:, sg * 512:(sg + 1) * 512],
                            lhsT=lhsT,
                            rhs=qkvwT[:, dc, sg * 512:(sg + 1) * 512],
                            start=(dc == 0),
                            stop=(dc == DC - 1),
                        )
                nc.vector.tensor_tensor(mu_all[:, tt, :], pmu, mub_bc[:],
                                        op=mybir.AluOpType.add)
                # evac/square/reduce as single 1536-wide ops, ALL on DVE:
                # no cross-engine hop inside the per-tile stats chain (the
                # bf16 square runs at the DVE 2x packed rate)
                stats = spool.tile([P, 24], F32, tag="st", name="stats")
                t = tpool.tile([P, 2, 1536], BF16, tag="t", name="tev")
                nc.vector.tensor_tensor(t[:, 0, :], psum[:], bias_bc[:, 0:1536],
                                        op=mybir.AluOpType.add)
                nc.vector.tensor_tensor(t[:, 1, :], t[:, 0, :], t[:, 0, :],
                                        op=mybir.AluOpType.mult)
                nc.vector.tensor_reduce(
                    stats[:],
                    t[:, 1, :].rearrange("p (g h) -> p g h", h=HD),
                    axis=mybir.AxisListType.X,
                    op=mybir.AluOpType.add,
                )
                rstd, m2 = emit_stats(tt, stats, 0, 24, 0, 12)
                for h in range(12):
                    emit_qnorm(tt, h, t[:, 0, h * HD:(h + 1) * HD],
                               rstd, m2, h, on_scalar=True)
                emit_kcenter(tt, 0, 1, t[:, 0, 768:1024], 12, rstd, m2, 12)
                emit_kcenter(tt, 1, 2, t[:, 0, 1024:1536], 16, rstd, m2, 16)

        # ---- per-head attention, with QKV pass B (heads 4-11 columns) and
        # the v projections interleaved to fill TensorE gaps ----
        with tc.tile_pool(name="ps_st", bufs=2, space="PSUM") as ps_st, \
             tc.tile_pool(name="ps_av", bufs=4, space="PSUM") as ps_av:

            def emit_v_chunk(vc):
                # v cols [1536+256*vc : 1536+256*(vc+1)] = heads 4vc..4vc+3
                c0 = 2 * D + 256 * vc
                for tt in range(NT):
                    pv = ps_av.tile([P, 512], F32, tag="av", name="ps_v")
                    for dc in range(DC):
                        nc.tensor.matmul(
                            pv[:, 0:256],
                            lhsT=xT[:, dc, tt * P:(tt + 1) * P],
                            rhs=qkvwT[:, dc, c0:c0 + 256],
                            start=(dc == 0),
                            stop=(dc == DC - 1),
                        )
                    hs = 4 * vc
                    nc.vector.tensor_tensor(
                        vext[:, tt, hs:hs + 4, 0:HD],
                        pv[:, 0:256].rearrange("p (s h) -> p s h", h=HD),
                        bias_bc[:, c0:c0 + 256].rearrange(
                            "p (s h) -> p s h", h=HD),
                        op=mybir.AluOpType.add,
                    )

            def emit_pair_transposes(hp):
                kkT = qkpool.tile([P, N], BF16, tag="kkT", name="kkT")
                qp0 = qkpool.tile([P, N], BF16, tag="qp0", name="qp0")
                qp1 = qkpool.tile([P, N], BF16, tag="qp1", name="qp1")
                nc.sync.dma_start_transpose(
                    kkT.rearrange("p (b t) -> p b t", t=P), knp[:, hp])
                nc.sync.dma_start_transpose(
                    qp0.rearrange("p (b t) -> p b t", t=P), qnp0[:, hp])
                nc.sync.dma_start_transpose(
                    qp1.rearrange("p (b t) -> p b t", t=P), qnp1[:, hp])
                return kkT, qp0, qp1

            def emit_head(h, kkT, qp0, qp1, prev):
                """Scores+exp for head h, 1:1 interleaved with the attn@v
                accumulation of head h-1 (prev)."""
                qT = qp0 if h % 2 == 0 else qp1
                E = epool.tile([P, NT, N], BF16, tag="E", name="E")
                if prev is not None:
                    hprev, Eprev = prev
                    pa0 = ps_av.tile([P, 512], F32, tag="av", name="pa0")
                    pa1 = ps_av.tile([P, 512], F32, tag="av", name="pa1")
                for jt in range(NT):
                    ps = ps_st.tile([P, N], F32, tag="st", name="ps_st_t")
                    for ic in range(2):
                        nc.tensor.matmul(
                            ps[:, ic * 512:(ic + 1) * 512],
                            lhsT=kkT[:, jt * P:(jt + 1) * P],
                            rhs=qT[:, ic * 512:(ic + 1) * 512],
                            start=True,
                            stop=True,
                        )
                    if apply_gn:
                        nc.scalar.activation(
                            E[:, jt, :], ps,
                            mybir.ActivationFunctionType.Exp, scale=SCALE)
                    else:
                        nc.scalar.activation(
                            E[:, jt, :], ps,
                            mybir.ActivationFunctionType.Exp,
                            scale=rks[:, jt, h:h + 1])
                    if prev is not None:
                        nc.tensor.matmul(
                            pa0, lhsT=vext[:, jt, hprev, :],
                            rhs=Eprev[:, jt, 0:512],
                            start=(jt == 0), stop=(jt == NT - 1),
                        )
                        nc.tensor.matmul(
                            pa1, lhsT=vext[:, jt, hprev, :],
                            rhs=Eprev[:, jt, 512:1024],
                            start=(jt == 0), stop=(jt == NT - 1),
                        )
                if prev is not None:
                    emit_normalize(hprev, pa0, pa1)
                return E

            def emit_av_tail(h, E):
                pa0 = ps_av.tile([P, 512], F32, tag="av", name="pa0")
                pa1 = ps_av.tile([P, 512], F32, tag="av", name="pa1")
                for jt in range(NT):
                    nc.tensor.matmul(
                        pa0, lhsT=vext[:, jt, h, :], rhs=E[:, jt, 0:512],
                        start=(jt == 0), stop=(jt == NT - 1),
                    )
                    nc.tensor.matmul(
                        pa1, lhsT=vext[:, jt, h, :], rhs=E[:, jt, 512:1024],
                        start=(jt == 0), stop=(jt == NT - 1),
                    )
                emit_normalize(h, pa0, pa1)

            def emit_normalize(h, pa0, pa1):
                for ic, pa in ((0, pa0), (1, pa1)):
                    rcp_t = nrm.tile([HD, 512], F32, tag="rcp_t", name="rcp_t")
                    s_sb = nrm.tile([HD, 512], F32, tag="s_sb", name="s_sb")
                    nc.vector.tensor_copy(s_sb[:], pa[HD:2 * HD, :])
                    nc.vector.reciprocal_approx_fast(rcp_t[:], s_sb[:])
                    nc.vector.tensor_tensor(
                        attnoutT[(h % 2) * HD:(h % 2 + 1) * HD, h // 2,
                                 ic * 512:(ic + 1) * 512],
                        pa[0:HD, :],
                        rcp_t[:],
                        op=mybir.AluOpType.mult,
                    )

            cur = emit_pair_transposes(0)
            emit_v_chunk(0)
            prev = None  # (h, E)
            for h in range(H):
                hp, hh = divmod(h, 2)
                if hh == 0 and hp > 0:
                    cur = nxt
                E = emit_head(h, *cur, prev)
                if h == 0:
                    emit_v_chunk(1)
                elif h == 1:
                    emit_v_chunk(2)
                if hh == 1 and hp + 1 < H // 2:
                    nxt = emit_pair_transposes(hp + 1)
                prev = (h, E)
            emit_av_tail(*prev)

        # ---- output projection: outT[e, t] = projwT.T @ attnoutT ----
        with tc.tile_pool(name="ps_pj", bufs=4, space="PSUM") as ps_pj:
            for et in range(DC):
                ps0 = ps_pj.tile([P, 512], F32, tag="pj", name="ps_pj0")
                ps1 = ps_pj.tile([P, 512], F32, tag="pj", name="ps_pj1")
                for oc in range(DC):
                    for tc2, ps in ((0, ps0), (1, ps1)):
                        nc.tensor.matmul(
                            ps,
                            lhsT=projwT[:, oc, et * P:(et + 1) * P],
                            rhs=attnoutT[:, oc, tc2 * 512:(tc2 + 1) * 512],
                            start=(oc == 0),
                            stop=(oc == DC - 1),
                        )
                for tc2, ps in ((0, ps0), (1, ps1)):
                    ot = outp.tile([P, 512], F32, tag="outt", name="ot")
                    nc.scalar.activation(
                        ot[:], ps, mybir.ActivationFunctionType.Identity,
                        bias=projb_col[:, et:et + 1],
                    )
                    nc.sync.dma_start(
                        out_d[et * P:(et + 1) * P, tc2 * 512:(tc2 + 1) * 512],
                        ot[:],
                    )

_NC_CACHE = {}


def _get_nc(apply_gn=True):
    if apply_gn not in _NC_CACHE:
        _NC_CACHE[apply_gn] = _build_graph(apply_gn)
    return _NC_CACHE[apply_gn]


def make_in_maps(x, qkv_w, qkv_b, proj_w, proj_b, qn_gamma, qn_beta):
    """Host-side layout prep: transpose + bf16-cast x and weight matrices so
    the kernel DMAs them straight into contraction-on-partitions layouts."""
    import ml_dtypes
    bf = ml_dtypes.bfloat16
    x = np.asarray(x, np.float32)
    qkv_w32 = np.asarray(qkv_w, np.float32)
    qkv_b32 = np.asarray(qkv_b, np.float32)
    # -mean of each 64-row head-group of qkv_w/qkv_b (24 q+k groups): the
    # kernel gets the LN means as extra matmul columns
    wsum = -qkv_w32[:1536].reshape(24, HD, D).mean(axis=1).T  # [D, 24]
    bsum = -qkv_b32[:1536].reshape(24, HD).mean(axis=1)
    shared = {
        "qkv_w": np.ascontiguousarray(qkv_w32.T.astype(bf)),
        "qkv_b": np.ascontiguousarray(qkv_b32),
        "qkv_bh": np.ascontiguousarray(qkv_b32.astype(bf)),
        "qkv_wsum": np.ascontiguousarray(wsum.astype(bf)),
        "qkv_bsum": np.ascontiguousarray(bsum.astype(np.float32)),
        "proj_w": np.ascontiguousarray(np.asarray(proj_w, np.float32).T.astype(bf)),
        "proj_b": np.ascontiguousarray(proj_b, np.float32),
        "qn_gamma": np.ascontiguousarray(qn_gamma, np.float32),
        "qn_beta": np.ascontiguousarray(qn_beta, np.float32),
    }
    return [
        {**shared, "x": np.ascontiguousarray(x[i].T.astype(bf))} for i in range(B)
    ]


def extract_output(res):
    return np.stack(
        [np.ascontiguousarray(res.results[i]["out"].T) for i in range(B)], axis=0
    )


def kernel(x, qkv_w, qkv_b, proj_w, proj_b, qn_gamma, qn_beta):
    qn_gamma = np.ascontiguousarray(qn_gamma, np.float32)
    qn_beta = np.ascontiguousarray(qn_beta, np.float32)
    apply_gn = not (np.all(qn_gamma == 1.0) and np.all(qn_beta == 0.0))
    nc = _get_nc(apply_gn)
    in_maps = make_in_maps(x, qkv_w, qkv_b, proj_w, proj_b, qn_gamma, qn_beta)
    res = run_bass_kernel_spmd(nc, in_maps, core_ids=list(range(B)))
    return extract_output(res)



# revision 8
# speedup vs baseline: 1.0234x; 1.0234x over previous
"""Multi-head attention forward on 8 TRN2 NeuronCores, data-parallel over batch.

Reference computation (per batch element b):
    qkv  = x @ qkv_w.T + qkv_b                     # [N, 3D]
    q, k = LN_headdim(q), LN_headdim(k)            # layernorm over head_dim=64
    S    = q @ k.T * hd^-0.5 ; A = softmax_j(S)    # per head
    out  = (A @ v) @ proj_w.T + proj_b             # [N, D]

v2 design (one batch element per core, no collectives), fully software-
pipelined so TensorE never waits for a phase boundary:
  - QKV is COLUMN-SLICED BY HEAD-PAIR: 6 pairs x 384 cols (q128|k128|v128,
    host-packed).  Pairs 0-1 prime the pipe; pair hp's 8 matmul groups ride
    inside head 2(hp-2)'s score stream, so the exp stream starts ~20us in
    and runs continuously to the end.  Group PSUM tiles ([P,384] f32, one
    bank) borrow the scores-tag rotation - PSUM stays within 8 banks
    (st [P,1024]x2 + av [P,512]x4).
  - k is NOT centered: against a fully-normalized q (sum_d q_n[d] = 0) the
    -mu_k term of k's layernorm vanishes in q_n.k; rstd_k folds into the
    exp's per-partition scale (scores^T has k-tokens on partitions), so k
    goes STRAIGHT from the QKV evacuation buffer into the block-transpose.
  - LN stats via one DVE bn_stats per (pair, tile); the per-pair rstd chain
    computes rstd = exp(-0.5 ln(var+eps)) on ScalarE - Ln/Exp/Identity all
    live in the SAME activation table as the softmax Exp, so the ACT table
    is never reloaded mid-stream (Sqrt would force a reload).
  - Scores computed TRANSPOSED: E = exp(scale_k * (k . q_n)) lands with
    k-tokens on partitions, directly the rhs of attn@v with V as lhsT.
  - q normalized into two zero-padded token-major buffers; plain 128x128
    block DMA transposes yield the K=128 zero-padded scores rhs.
  - Softmax denominators via 64 ones-columns in V (PSUM rows 64:128 hold
    the sums); normalize = SBUF copy + reciprocal_approx_fast + one VectorE
    multiply writing attnoutT.  (reciprocal_approx_fast must NOT read PSUM
    directly: it returns garbage on HW while passing in CoreSim.)
  - Heads software-pipelined 1:1 (scores of h interleave attn@v of h-1);
    projection computes outT = projwT.T @ attnoutT at the tail; ScalarE
    (idle after exps) applies the bias during PSUM evacuation; host flips.
"""

import os
import sys

import numpy as np

sys.path.insert(0, "/opt/trn_rl_repo")

from contextlib import ExitStack

import concourse.bass as bass
import concourse.tile as tile
from concourse import bacc, mybir
from concourse.bass_utils import run_bass_kernel_spmd

B, N, D = 8, 1024, 768
H, HD = 12, 64
NP = H // 2        # 6 head pairs
P = 128
NT = N // P        # 8 token tiles
DC = D // P        # 6 contraction subtiles
GC = 3 * P         # 384 qkv columns per pair group (q|k|v)
EPS = 1e-5
SCALE = HD ** -0.5  # 0.125
F32 = mybir.dt.float32
BF16 = mybir.dt.bfloat16


def _bcast_ap(ap_1d, parts):
    """View a 1-D DRAM AP as [parts, n] with partition stride 0 (broadcast)."""
    return bass.AP(
        tensor=ap_1d.tensor,
        offset=ap_1d.offset,
        ap=[[0, parts]] + list(ap_1d.ap),
    )


def _build_graph(apply_gn):
    nc = bacc.Bacc("TRN2", target_bir_lowering=False, debug=False, num_devices=B)

    x_d = nc.dram_tensor("x", [D, N], BF16, kind="ExternalInput").ap()
    # host-packed per-pair qkv weights: [D, pair*384] with 384 = q|k|v cols
    qkvw_d = nc.dram_tensor("qkv_wp", [D, NP * GC], BF16, kind="ExternalInput").ap()
    # host-packed biases: per-pair q|k (256 cols), per-head v (64)
    bqk_d = nc.dram_tensor("qkv_bqk", [NP * 2 * P], BF16, kind="ExternalInput").ap()
    bv_d = nc.dram_tensor("qkv_bv", [H * HD], BF16, kind="ExternalInput").ap()
    projw_d = nc.dram_tensor("proj_w", [D, D], BF16, kind="ExternalInput").ap()
    projb_d = nc.dram_tensor("proj_b", [D], F32, kind="ExternalInput").ap()
    gamma_d = nc.dram_tensor("qn_gamma", [HD], F32, kind="ExternalInput").ap()
    beta_d = nc.dram_tensor("qn_beta", [HD], F32, kind="ExternalInput").ap()
    # output is produced TRANSPOSED ([e, t]); the host flips it back
    out_d = nc.dram_tensor("out", [D, N], F32, kind="ExternalOutput").ap()

    with tile.TileContext(nc) as tc:
        _emit(tc, out_d, x_d, qkvw_d, bqk_d, bv_d, projw_d, projb_d,
              gamma_d, beta_d, apply_gn)

    nc.compile()
    return nc


def _emit(tc, out_d, x_d, qkvw_d, bqk_d, bv_d, projw_d, projb_d,
          gamma_d, beta_d, apply_gn):
    nc = tc.nc
    ctx = ExitStack()
    with ctx:
        const = ctx.enter_context(tc.tile_pool(name="const", bufs=1))
        wpool = ctx.enter_context(tc.tile_pool(name="wts", bufs=1))
        data = ctx.enter_context(tc.tile_pool(name="data", bufs=1))
        epool = ctx.enter_context(tc.tile_pool(name="escore", bufs=2))
        qkpool = ctx.enter_context(tc.tile_pool(name="qk", bufs=2))
        spool = ctx.enter_context(tc.tile_pool(name="stats", bufs=2))
        outp = ctx.enter_context(tc.tile_pool(name="outp", bufs=3))
        nrm = ctx.enter_context(tc.tile_pool(name="nrm", bufs=2))
        ps = ctx.enter_context(tc.tile_pool(name="ps", bufs=1, space="PSUM"))

        # ---- weight / input DMAs, interleaved so pair-0 work can start
        # as early as possible: x[dc] + pair-0 weights first ----
        xT = wpool.tile([P, DC, N], BF16)            # [d_in, dc, t]
        qkvwT = wpool.tile([P, DC, NP, GC], BF16)    # [d_in, dc, pair, 384]
        projwT = wpool.tile([P, DC, D], BF16)        # [o_in, oc, e]
        x_r = x_d.rearrange("(dc p) t -> p dc t", p=P)
        w_r = qkvw_d.rearrange("(dc p) x -> p dc x", p=P)
        for dc in range(DC):
            nc.sync.dma_start(xT[:, dc, :], x_r[:, dc, :])
            nc.sync.dma_start(qkvwT[:, dc, 0, :], w_r[:, dc, 0:GC])
        for dc in range(DC):
            nc.sync.dma_start(qkvwT[:, dc, 1, :], w_r[:, dc, GC:2 * GC])

        # broadcast constants (after the first-needed matmul operands)
        bqk_bc = const.tile([P, NP, 2 * P], BF16)
        nc.sync.dma_start(bqk_bc[:], _bcast_ap(bqk_d, P))
        bv_bc = const.tile([P, H, HD], BF16)
        nc.sync.dma_start(bv_bc[:], _bcast_ap(bv_d, P))
        projb_col = const.tile([P, DC], F32)
        nc.sync.dma_start(projb_col[:], projb_d.rearrange("(et p) -> p et", p=P))
        if apply_gn:
            gamma_bc = const.tile([P, HD], F32)
            nc.sync.dma_start(gamma_bc[:], _bcast_ap(gamma_d, P))
            beta_bc = const.tile([P, HD], F32)
            nc.sync.dma_start(beta_bc[:], _bcast_ap(beta_d, P))

        for hp in range(2, NP):
            for dc in range(DC):
                nc.sync.dma_start(
                    qkvwT[:, dc, hp, :], w_r[:, dc, hp * GC:(hp + 1) * GC]
                )
        for dc in range(DC):
            nc.sync.dma_start(
                projwT[:, dc, :],
                projw_d.rearrange("(dc p) e -> p dc e", p=P)[:, dc, :],
            )

        # ---- persistent SBUF data tiles ----
        # qkv evacuations (per pair, double-buffered by pair parity); q and k
        # in separate tiles so the k block-transpose source is 2D-contiguous.
        # cols 0:64 = even head, 64:128 = odd head
        q_ev = data.tile([P, 2, NT, P], BF16)
        k_ev = data.tile([P, 2, NT, P], BF16)
        # q normalized, token-major, zero-padded halves (for DMA transpose)
        qnp0 = data.tile([P, 2, NT, P], BF16)   # cols 0:64 = q even head
        qnp1 = data.tile([P, 2, NT, P], BF16)   # cols 64:128 = q odd head
        nc.vector.memset(qnp0[:, :, :, HD:2 * HD], 0.0)
        nc.vector.memset(qnp1[:, :, :, 0:HD], 0.0)
        # v with 64 ones-columns: attn@v psum rows 64:128 = softmax denoms
        vext = data.tile([P, NT, H, 2 * HD], BF16)
        nc.vector.memset(vext[:, :, :, HD:2 * HD], 1.0)
        # attnoutT [o_in, oc, t] written by the normalize step
        attnoutT = data.tile([P, DC, N], BF16)
        # 0.125 * rstd_k per (token-tile, head): per-partition exp scales
        rks = data.tile([P, NT, H], F32)
        # bn_stats output per pair: [P, parity, tt, 4 groups, 6]
        bnout = data.tile([P, 2, NT, 4, 6], F32)
        # per-pair q-norm params [P, parity, tt, grp] (0=q_even 1=q_odd)
        rstdq = data.tile([P, 2, NT, 2], F32)
        m2q = data.tile([P, 2, NT, 2], F32)
        if apply_gn:
            rstdk = data.tile([P, 2, NT, 2], F32)
            m2k = data.tile([P, 2, NT, 2], F32)

        # ---------------- emission helpers ----------------
        def st_tile():
            return ps.tile([P, N], F32, tag="st", name="ps_st", bufs=2)

        def av_tile():
            return ps.tile([P, 512], F32, tag="av", name="ps_av", bufs=4)

        def emit_group(hp, tt):
            """One QKV matmul group: psum[:, 0:384] = x_tt @ w_pair_hp,
            then evacuations (q|k to qk_ev, v to vext) and bn_stats."""
            pg = st_tile()
            for dc in range(DC):
                nc.tensor.matmul(
                    pg[:, 0:GC],
                    lhsT=xT[:, dc, tt * P:(tt + 1) * P],
                    rhs=qkvwT[:, dc, hp, :],
                    start=(dc == 0),
                    stop=(dc == DC - 1),
                )
            pb = hp % 2
            nc.vector.tensor_tensor(
                q_ev[:, pb, tt, :], pg[:, 0:P], bqk_bc[:, hp, 0:P],
                op=mybir.AluOpType.add,
            )
            nc.vector.tensor_tensor(
                k_ev[:, pb, tt, :], pg[:, P:2 * P], bqk_bc[:, hp, P:2 * P],
                op=mybir.AluOpType.add,
            )
            nc.vector.tensor_tensor(
                vext[:, tt, 2 * hp:2 * hp + 2, 0:HD],
                pg[:, 2 * P:3 * P].rearrange("p (s h) -> p s h", h=HD),
                bv_bc[:, 2 * hp:2 * hp + 2, :],
                op=mybir.AluOpType.add,
            )
            # HW restriction: one bn_stats = one 6-element output group
            for par in range(2):
                nc.vector.bn_stats(
                    bnout[:, pb, tt, par],
                    q_ev[:, pb, tt, par * HD:(par + 1) * HD],
                )
                nc.vector.bn_stats(
                    bnout[:, pb, tt, 2 + par],
                    k_ev[:, pb, tt, par * HD:(par + 1) * HD],
                )

        def emit_pair_stats(hp):
            """Per-pair rstd/m2 chain from bnout; fills rstdq/m2q/rks.
            bn_stats gives per group: [cnt_e, mean_e, M2_e, cnt_o, mean_o,
            M2_o] over even/odd elements.  mean = (me+mo)/2 and
            var = (M2e+M2o)/64 + ((me-mo)/2)^2."""
            pb = hp % 2
            me = bnout[:, pb, :, :, 1]    # [P, NT, 4]
            mo = bnout[:, pb, :, :, 4]
            M2e = bnout[:, pb, :, :, 2]
            M2o = bnout[:, pb, :, :, 5]
            a = spool.tile([P, NT, 4], F32, tag="a", name="sa")
            d = spool.tile([P, NT, 4], F32, tag="d", name="sd")
            var = spool.tile([P, NT, 4], F32, tag="var", name="svar")
            mu = spool.tile([P, NT, 4], F32, tag="mu", name="smu")
            rst = spool.tile([P, NT, 4], F32, tag="rst", name="srst")
            nc.vector.tensor_tensor(a, M2e, M2o, op=mybir.AluOpType.add)
            nc.vector.tensor_tensor(d, me, mo, op=mybir.AluOpType.subtract)
            nc.vector.tensor_tensor(d, d, d, op=mybir.AluOpType.mult)
            nc.vector.tensor_scalar(a, a, 1.0 / HD, EPS,
                                    op0=mybir.AluOpType.mult,
                                    op1=mybir.AluOpType.add)
            nc.vector.tensor_scalar(d, d, 0.25, 0.0,
                                    op0=mybir.AluOpType.mult,
                                    op1=mybir.AluOpType.add)
            nc.vector.tensor_tensor(var, a, d, op=mybir.AluOpType.add)
            # rstd = exp(-0.5 * ln(var+eps)); Ln/Exp share the act table
            nc.scalar.activation(a, var, mybir.ActivationFunctionType.Ln)
            nc.scalar.activation(rst, a, mybir.ActivationFunctionType.Exp,
                                 scale=-0.5)
            # m2 = -mean * rstd  (qnorm per-partition bias)
            nc.vector.tensor_tensor(mu, me, mo, op=mybir.AluOpType.add)
            nc.vector.tensor_tensor(mu, mu, rst, op=mybir.AluOpType.mult)
            nc.vector.tensor_scalar(mu, mu, -0.5, 0.0,
                                    op0=mybir.AluOpType.mult,
                                    op1=mybir.AluOpType.add)
            nc.vector.tensor_copy(rstdq[:, pb], rst[:, :, 0:2])
            nc.vector.tensor_copy(m2q[:, pb], mu[:, :, 0:2])
            if not apply_gn:
                # k rstd -> exp scale table (0.125 * rstd_k)
                nc.vector.tensor_scalar(
                    rks[:, :, 2 * hp:2 * hp + 2], rst[:, :, 2:4], SCALE, 0.0,
                    op0=mybir.AluOpType.mult, op1=mybir.AluOpType.add)
            else:
                nc.vector.tensor_copy(rstdk[:, pb], rst[:, :, 2:4])
                nc.vector.tensor_copy(m2k[:, pb], mu[:, :, 2:4])

        def emit_pair_norms(hp):
            """q normalize into qnp0/qnp1 (+ for gn: full k LN in place)."""
            pb = hp % 2
            for tt in range(NT):
                for par in range(2):
                    dst = (qnp1[:, pb, tt, HD:2 * HD] if par
                           else qnp0[:, pb, tt, 0:HD])
                    nc.vector.tensor_scalar(
                        dst, q_ev[:, pb, tt, par * HD:(par + 1) * HD],
                        rstdq[:, pb, tt, par:par + 1],
                        m2q[:, pb, tt, par:par + 1],
                        op0=mybir.AluOpType.mult, op1=mybir.AluOpType.add)
                    if apply_gn:
                        nc.gpsimd.tensor_tensor(dst, dst, gamma_bc[:, 0:HD],
                                                op=mybir.AluOpType.mult)
                        nc.gpsimd.tensor_tensor(dst, dst, beta_bc[:, 0:HD],
                                                op=mybir.AluOpType.add)
                        kd = k_ev[:, pb, tt, par * HD:(par + 1) * HD]
                        nc.vector.tensor_scalar(
                            kd, kd,
                            rstdk[:, pb, tt, par:par + 1],
                            m2k[:, pb, tt, par:par + 1],
                            op0=mybir.AluOpType.mult, op1=mybir.AluOpType.add)
                        nc.gpsimd.tensor_tensor(kd, kd, gamma_bc[:, 0:HD],
                                                op=mybir.AluOpType.mult)
                        nc.gpsimd.tensor_tensor(kd, kd, beta_bc[:, 0:HD],
                                                op=mybir.AluOpType.add)

        def emit_pair_transposes(hp):
            pb = hp % 2
            kkT = qkpool.tile([P, N], BF16, tag="kkT", name="kkT")
            qp0 = qkpool.tile([P, N], BF16, tag="qp0", name="qp0")
            qp1 = qkpool.tile([P, N], BF16, tag="qp1", name="qp1")
            nc.sync.dma_start_transpose(
                kkT.rearrange("p (b t) -> p b t", t=P), k_ev[:, pb])
            nc.sync.dma_start_transpose(
                qp0.rearrange("p (b t) -> p b t", t=P), qnp0[:, pb])
            nc.sync.dma_start_transpose(
                qp1.rearrange("p (b t) -> p b t", t=P), qnp1[:, pb])
            return kkT, qp0, qp1

        def emit_normalize(h, pa0, pa1):
            for ic, pa in ((0, pa0), (1, pa1)):
                rcp_t = nrm.tile([HD, 512], F32, tag="rcp_t", name="rcp_t")
                s_sb = nrm.tile([HD, 512], F32, tag="s_sb", name="s_sb")
                nc.vector.tensor_copy(s_sb[:], pa[HD:2 * HD, :])
                nc.vector.reciprocal_approx_fast(rcp_t[:], s_sb[:])
                nc.vector.tensor_tensor(
                    attnoutT[(h % 2) * HD:(h % 2 + 1) * HD, h // 2,
                             ic * 512:(ic + 1) * 512],
                    pa[0:HD, :],
                    rcp_t[:],
                    op=mybir.AluOpType.mult,
                )

        def emit_head(h, kkT, qp0, qp1, prev, gsrc):
            """Scores+exp for head h, 1:1 interleaved with the attn@v of
            head h-1 (prev), plus one QKV group of pair gsrc per jt slot."""
            qT = qp0 if h % 2 == 0 else qp1
            E = epool.tile([P, NT, N], BF16, tag="E", name="E")
            if prev is not None:
                hprev, Eprev = prev
                pa0 = av_tile()
                pa1 = av_tile()
            for jt in range(NT):
                pst = st_tile()
                for ic in range(2):
                    nc.tensor.matmul(
                        pst[:, ic * 512:(ic + 1) * 512],
                        lhsT=kkT[:, jt * P:(jt + 1) * P],
                        rhs=qT[:, ic * 512:(ic + 1) * 512],
                        start=True,
                        stop=True,
                    )
                if apply_gn:
                    nc.scalar.activation(
                        E[:, jt, :], pst,
                        mybir.ActivationFunctionType.Exp, scale=SCALE)
                else:
                    nc.scalar.activation(
                        E[:, jt, :], pst,
                        mybir.ActivationFunctionType.Exp,
                        scale=rks[:, jt, h:h + 1])
                if prev is not None:
                    nc.tensor.matmul(
                        pa0, lhsT=vext[:, jt, hprev, :],
                        rhs=Eprev[:, jt, 0:512],
                        start=(jt == 0), stop=(jt == NT - 1),
                    )
                    nc.tensor.matmul(
                        pa1, lhsT=vext[:, jt, hprev, :],
                        rhs=Eprev[:, jt, 512:1024],
                        start=(jt == 0), stop=(jt == NT - 1),
                    )
                if gsrc is not None:
                    emit_group(gsrc, jt)
            if prev is not None:
                emit_normalize(hprev, pa0, pa1)
            return E

        def emit_av_tail(h, E):
            pa0 = av_tile()
            pa1 = av_tile()
            for jt in range(NT):
                nc.tensor.matmul(
                    pa0, lhsT=vext[:, jt, h, :], rhs=E[:, jt, 0:512],
                    start=(jt == 0), stop=(jt == NT - 1),
                )
                nc.tensor.matmul(
                    pa1, lhsT=vext[:, jt, h, :], rhs=E[:, jt, 512:1024],
                    start=(jt == 0), stop=(jt == NT - 1),
                )
            emit_normalize(h, pa0, pa1)

        # ---------------- the pipeline ----------------
        # prime: pairs 0 and 1
        for tt in range(NT):
            emit_group(0, tt)
        emit_pair_stats(0)
        emit_pair_norms(0)
        cur = emit_pair_transposes(0)
        for tt in range(NT):
            emit_group(1, tt)
        emit_pair_stats(1)
        emit_pair_norms(1)

        prev = None
        nxt = None
        for h in range(H):
            hp, hh = divmod(h, 2)
            if hh == 0 and hp > 0:
                cur = nxt
            # during even heads: one QKV group of pair hp+2 per jt slot
            gsrc = hp + 2 if (hh == 0 and hp + 2 < NP) else None
            E = emit_head(h, *cur, prev, gsrc)
            if gsrc is not None:
                emit_pair_stats(gsrc)
                emit_pair_norms(gsrc)
            if hh == 0 and hp + 1 < NP:
                nxt = emit_pair_transposes(hp + 1)
            prev = (h, E)
        emit_av_tail(*prev)

        # ---- output projection: outT[e, t] = projwT.T @ attnoutT ----
        for et in range(DC):
            ps0 = av_tile()
            ps1 = av_tile()
            for oc in range(DC):
                for half, pp in ((0, ps0), (1, ps1)):
                    nc.tensor.matmul(
                        pp,
                        lhsT=projwT[:, oc, et * P:(et + 1) * P],
                        rhs=attnoutT[:, oc, half * 512:(half + 1) * 512],
                        start=(oc == 0),
                        stop=(oc == DC - 1),
                    )
            for half, pp in ((0, ps0), (1, ps1)):
                ot = outp.tile([P, 512], F32, tag="outt", name="ot")
                nc.scalar.activation(
                    ot[:], pp, mybir.ActivationFunctionType.Identity,
                    bias=projb_col[:, et:et + 1],
                )
                nc.sync.dma_start(
                    out_d[et * P:(et + 1) * P, half * 512:(half + 1) * 512],
                    ot[:],
                )


_NC_CACHE = {}


def _get_nc(apply_gn=True):
    if apply_gn not in _NC_CACHE:
        _NC_CACHE[apply_gn] = _build_graph(apply_gn)
    return _NC_CACHE[apply_gn]


def make_in_maps(x, qkv_w, qkv_b, proj_w, proj_b, qn_gamma, qn_beta):
    """Host-side layout prep: transpose + bf16-cast x / weights; pack qkv
    weights and biases by head pair (q128|k128|v128 columns per pair)."""
    import ml_dtypes
    bf = ml_dtypes.bfloat16
    x = np.asarray(x, np.float32)
    qkv_w32 = np.asarray(qkv_w, np.float32)
    qkv_b32 = np.asarray(qkv_b, np.float32)
    wT = qkv_w32.T  # [D, 3D]: rows of qkv_w = out dims q|k|v
    pair_w = np.concatenate(
        [
            np.concatenate(
                [
                    wT[:, P * hp:P * (hp + 1)],
                    wT[:, D + P * hp:D + P * (hp + 1)],
                    wT[:, 2 * D + P * hp:2 * D + P * (hp + 1)],
                ],
                axis=1,
            )
            for hp in range(NP)
        ],
        axis=1,
    )  # [D, 6*384]
    bqk = np.concatenate(
        [
            np.concatenate(
                [qkv_b32[P * hp:P * (hp + 1)],
                 qkv_b32[D + P * hp:D + P * (hp + 1)]]
            )
            for hp in range(NP)
        ]
    )  # [6*256]
    bv = qkv_b32[2 * D:]  # [768] per-head v bias
    shared = {
        "qkv_wp": np.ascontiguousarray(pair_w.astype(bf)),
        "qkv_bqk": np.ascontiguousarray(bqk.astype(bf)),
        "qkv_bv": np.ascontiguousarray(bv.astype(bf)),
        "proj_w": np.ascontiguousarray(np.asarray(proj_w, np.float32).T.astype(bf)),
        "proj_b": np.ascontiguousarray(proj_b, np.float32),
        "qn_gamma": np.ascontiguousarray(qn_gamma, np.float32),
        "qn_beta": np.ascontiguousarray(qn_beta, np.float32),
    }
    return [
        {**shared, "x": np.ascontiguousarray(x[i].T.astype(bf))} for i in range(B)
    ]


def extract_output(res):
    return np.stack(
        [np.ascontiguousarray(res.results[i]["out"].T) for i in range(B)], axis=0
    )


def kernel(x, qkv_w, qkv_b, proj_w, proj_b, qn_gamma, qn_beta):
    qn_gamma = np.ascontiguousarray(qn_gamma, np.float32)
    qn_beta = np.ascontiguousarray(qn_beta, np.float32)
    apply_gn = not (np.all(qn_gamma == 1.0) and np.all(qn_beta == 0.0))
    nc = _get_nc(apply_gn)
    in_maps = make_in_maps(x, qkv_w, qkv_b, proj_w, proj_b, qn_gamma, qn_beta)
    res = run_bass_kernel_spmd(nc, in_maps, core_ids=list(range(B)))
    return extract_output(res)
